# revision 7
# baseline (speedup 1.0000x reference)
"""Trainium2 Bass kernel for modulated-RMSNorm + 2D-RoPE multi-head attention.

Shards batch 16 -> 8 cores x 2 batches. Per core, per batch:
  modT = mod_w @ t.T (feature-major), A1 = 1+sc, B' = sh
  xA   = xT * A1                       (feature-major, f32r)
  rstd = rsqrt(mean(x^2)+eps)          (PE ones-row matvec on xT^2)
  qkT  = (Wqk_t.T @ xA) * rstd + bias  (feature-major, rope'd in place)
  v    = (xA.T @ Wv_t) * rstd          (token-major, ones column appended)
  S.T  = kT.T @ qT per head (two K=32 accumulating matmuls; rope row split)
  PT   = exp(0.125 * S.T)              (ACT, f32r)
  OT   = (v_ext.T @ PT)[0:64] * recip(rowsum)   (feature-major)
  out  = OT.T @ woT + ones.T @ (b_v @ woT)      (K=1 bias matmul)
All heavy matmuls run in float32r (full PE rate at N=512).
"""
import numpy as np
import concourse.mybir as mybir
import concourse.tile as tile
from concourse import bacc
from concourse.bass_utils import run_bass_kernel_spmd

F32 = mybir.dt.float32
F32R = mybir.dt.float32r
EXP = mybir.ActivationFunctionType.Exp
SQRT = mybir.ActivationFunctionType.Sqrt
MULT = mybir.AluOpType.mult

HEADS, HD, DIM, NTOK, B, NCORES = 16, 64, 1024, 1024, 16, 8
BPC = B // NCORES          # batches per core
DC = DIM // 128            # dim chunks
TT = NTOK // 128           # token tiles
EPS = 1e-6

TRACE = False
LAST_EXEC_NS = None

_CACHE = {}


def _build():
    nc = bacc.Bacc("TRN2", target_bir_lowering=False, debug=False)
    xT_d = nc.declare_dram_parameter("xT", [BPC, DIM, NTOK], F32, isOutput=False)
    tT_d = nc.declare_dram_parameter("tT", [DIM, BPC], F32R, isOutput=False)
    wqk_d = nc.declare_dram_parameter("wqk", [DIM, 2048], F32R, isOutput=False)
    wv_d = nc.declare_dram_parameter("wv", [DIM, 1024], F32R, isOutput=False)
    wo_d = nc.declare_dram_parameter("wo", [DIM, 1024], F32R, isOutput=False)
    mw_d = nc.declare_dram_parameter("mw", [DIM, 2048], F32R, isOutput=False)
    cos_d = nc.declare_dram_parameter("cos4", [128, NTOK], F32, isOutput=False)
    sin_d = nc.declare_dram_parameter("sin4", [128, NTOK], F32, isOutput=False)
    out_d = nc.declare_dram_parameter("out", [BPC, NTOK, DIM], F32, isOutput=True)
    rsc_d = nc.declare_dram_parameter("rsc", [BPC, NTOK], F32, isOutput=True)
    bsc_d = nc.declare_dram_parameter("bsc", [2, 2, 512], F32R, isOutput=True)

    with tile.TileContext(nc) as tc:
        with tc.tile_pool(name="const", bufs=1) as cp:
            cos4 = cp.tile([128, NTOK], F32, tag="cos4")
            sin4 = cp.tile([128, NTOK], F32, tag="sin4")
            for tqc in range(2):
                nc.sync.dma_start(out=cos4[:, 512 * tqc:512 * (tqc + 1)],
                                  in_=cos_d[:, 512 * tqc:512 * (tqc + 1)])
                nc.sync.dma_start(out=sin4[:, 512 * tqc:512 * (tqc + 1)],
                                  in_=sin_d[:, 512 * tqc:512 * (tqc + 1)])
            tT_sb = cp.tile([128, DC, BPC], F32R, tag="tT")
            for kc in range(DC):
                nc.sync.dma_start(out=tT_sb[:, kc, :],
                                  in_=tT_d[128 * kc:128 * (kc + 1), :])
            modT = cp.tile([128, 16, BPC], F32R, tag="modT")
            A1 = cp.tile([128, DC, BPC], F32, tag="A1")
            qkvb = cp.tile([128, 16, BPC], F32, tag="qkvb")
            qkvb_v = cp.tile([128, 8, BPC], F32R, tag="qkvb_v")
            ones_c = cp.tile([128, 1], F32R, tag="ones_c")      # ssq lhsT
            ones_r = cp.tile([1, 128], F32R, tag="ones_r")      # K=1 bias mm lhsT
            ones_v = cp.tile([128, 128], F32, tag="ones_v")     # v ones column src
            nc.vector.memset(ones_v, 1.0)
            nc.vector.tensor_copy(ones_c, ones_v[:, 0:1])
            nc.vector.tensor_copy(ones_r, ones_v[0:1, :])
            bias_ev = cp.tile([2, 2, 512], F32R, tag="bias_ev")
            bias_row = [cp.tile([1, NTOK], F32R, tag=f"bias_row{b}",
                                name=f"bias_row{b}") for b in range(BPC)]
            rstd_rep = cp.tile([128, NTOK], F32, tag="rstd_rep")
            eps_t = cp.tile([1, 1], F32, tag="eps_t")
            nc.vector.memset(eps_t, EPS)
            rstd_tm = cp.tile([128, TT], F32, tag="rstd_tm")

            # ---- phase A: modT, A1, qkv bias, bias_out ----
            with tc.tile_pool(name="pha", bufs=6) as pa, \
                 tc.tile_pool(name="pha_wo", bufs=4) as pawo, \
                 tc.tile_pool(name="psA", bufs=2, space="PSUM") as psA:
                for mc in range(16):
                    ps = psA.tile([128, BPC], F32, tag="pm")
                    for kc in range(DC):
                        wt = pa.tile([128, 128], F32R, tag="mw")
                        nc.sync.dma_start(
                            out=wt, in_=mw_d[128 * kc:128 * (kc + 1),
                                             128 * mc:128 * (mc + 1)])
                        nc.tensor.matmul(ps, wt, tT_sb[:, kc, :],
                                         start=(kc == 0), stop=(kc == DC - 1))
                    nc.vector.tensor_copy(modT[:, mc, :], ps)
                nc.vector.tensor_scalar_add(out=A1, in0=modT[:, 0:8, :],
                                            scalar1=1.0)
                # qkv bias: qkvb[j, b] = sum_d B'[d, b] * W_t[d, j]
                for mc in range(24):
                    src = wqk_d if mc < 16 else wv_d
                    col = 128 * mc if mc < 16 else 128 * (mc - 16)
                    ps = psA.tile([128, BPC], F32, tag="pb")
                    for kc in range(DC):
                        wt = pa.tile([128, 128], F32R, tag="wb")
                        nc.sync.dma_start(
                            out=wt, in_=src[128 * kc:128 * (kc + 1),
                                            col:col + 128])
                        nc.tensor.matmul(ps, wt, modT[:, 8 + kc, :],
                                         start=(kc == 0), stop=(kc == DC - 1))
                    if mc < 16:
                        nc.vector.tensor_copy(qkvb[:, mc, :], ps)
                    else:
                        nc.vector.tensor_copy(qkvb_v[:, mc - 16, :], ps)
                # bias_out[b, :] = b_v[:, b] @ woT
                for doutc in range(2):
                    ps = psA.tile([BPC, 512], F32, tag="pbo")
                    for jc in range(8):
                        wt = pawo.tile([128, 512], F32R, tag="wo")
                        nc.sync.dma_start(
                            out=wt, in_=wo_d[128 * jc:128 * (jc + 1),
                                             512 * doutc:512 * (doutc + 1)])
                        nc.tensor.matmul(ps, qkvb_v[:, jc, :], wt,
                                         start=(jc == 0), stop=(jc == 7))
                    nc.vector.tensor_copy(bias_ev[:, doutc, :], ps)
                nc.sync.dma_start(out=bsc_d[:], in_=bias_ev)
                for b in range(BPC):
                    nc.sync.dma_start(
                        out=bias_row[b],
                        in_=bsc_d[b:b + 1, :, :].rearrange("o a n -> o (a n)"))

            # ---- per-batch ----
            for b in range(BPC):
                with tc.tile_pool(name=f"qv{b}", bufs=1) as qv:
                    qk_sb = qv.tile([128, 16, NTOK], F32R, tag="qk")
                    v_sb = qv.tile([128, TT, HEADS, HD + 1], F32R, tag="v")
                    with tc.tile_pool(name=f"ph2_{b}", bufs=1) as p2, \
                         tc.tile_pool(name=f"xt{b}", bufs=2) as pxt, \
                         tc.tile_pool(name=f"wq{b}", bufs=10) as pwq, \
                         tc.tile_pool(name=f"wv{b}", bufs=3) as pwv:
                        xA = p2.tile([128, DC, NTOK], F32R, tag="xA")
                        rrow = p2.tile([1, NTOK], F32, tag="rrow")
                        rrow2 = p2.tile([1, NTOK], F32, tag="rrow2")
                        # ssq + xA
                        with tc.tile_pool(name=f"pss{b}", bufs=2,
                                          space="PSUM") as pss:
                            ps_s = [pss.tile([1, 512], F32, tag="ss",
                                             name=f"ssq{b}_{i}")
                                    for i in range(2)]
                            for kc in range(DC):
                                xt = pxt.tile([128, NTOK], F32, tag="xt")
                                nc.sync.dma_start(
                                    out=xt, in_=xT_d[b, 128 * kc:128 * (kc + 1), :])
                                xsq = pxt.tile([128, NTOK], F32R, tag="xsq")
                                nc.vector.tensor_mul(xsq, xt, xt)
                                for tqc in range(2):
                                    nc.tensor.matmul(
                                        ps_s[tqc], ones_c,
                                        xsq[:, 512 * tqc:512 * (tqc + 1)],
                                        start=(kc == 0), stop=(kc == DC - 1))
                                nc.vector.tensor_scalar_mul(
                                    out=xA[:, kc, :], in0=xt,
                                    scalar1=A1[:, kc, b:b + 1])
                            for tqc in range(2):
                                nc.scalar.activation(
                                    out=rrow[:, 512 * tqc:512 * (tqc + 1)],
                                    in_=ps_s[tqc], func=SQRT,
                                    scale=1.0 / DIM, bias=eps_t[:, 0:1])
                        nc.vector.reciprocal(out=rrow2, in_=rrow)
                        nc.gpsimd.partition_broadcast(rstd_rep, rrow2)
                        nc.sync.dma_start(out=rsc_d[b:b + 1, :], in_=rrow2)
                        nc.sync.dma_start(
                            out=rstd_tm,
                            in_=rsc_d[b:b + 1, :].rearrange(
                                "o (t p) -> (o p) t", p=128))

                        # v matmuls (token-major)
                        with tc.tile_pool(name=f"psv{b}", bufs=8,
                                          space="PSUM") as psv:
                            for nch in range(2):
                                ps_v = [psv.tile([128, 512], F32, tag="v",
                                                 name=f"psv{b}_{nch}_{i}")
                                        for i in range(TT)]
                                for kc in range(DC):
                                    wt = pwv.tile([128, 512], F32R, tag="wv")
                                    nc.sync.dma_start(
                                        out=wt,
                                        in_=wv_d[128 * kc:128 * (kc + 1),
                                                 512 * nch:512 * (nch + 1)])
                                    for tt in range(TT):
                                        nc.tensor.matmul(
                                            ps_v[tt],
                                            xA[:, kc, 128 * tt:128 * (tt + 1)],
                                            wt, start=(kc == 0),
                                            stop=(kc == DC - 1))
                                for tt in range(TT):
                                    nc.vector.tensor_scalar_mul(
                                        out=v_sb[:, tt, 8 * nch:8 * (nch + 1), 0:HD],
                                        in0=ps_v[tt].rearrange(
                                            "p (h d) -> p h d", d=HD),
                                        scalar1=rstd_tm[:, tt:tt + 1])
                        nc.vector.tensor_copy(
                            out=v_sb[:, :, :, HD],
                            in_=ones_v.rearrange("p (a h) -> p a h", a=TT))

                        # qk matmuls (feature-major) + eviction
                        with tc.tile_pool(name=f"psq{b}", bufs=4,
                                          space="PSUM") as psq:
                            for mc in range(16):
                                wts = []
                                for kc in range(DC):
                                    wt = pwq.tile([128, 128], F32R, tag="wqk")
                                    nc.sync.dma_start(
                                        out=wt,
                                        in_=wqk_d[128 * kc:128 * (kc + 1),
                                                  128 * mc:128 * (mc + 1)])
                                    wts.append(wt)
                                for tqc in range(2):
                                    sl = slice(512 * tqc, 512 * (tqc + 1))
                                    ps = psq.tile([128, 512], F32, tag="qk")
                                    for kc in range(DC):
                                        nc.tensor.matmul(
                                            ps, wts[kc], xA[:, kc, sl],
                                            start=(kc == 0), stop=(kc == DC - 1))
                                    nc.vector.tensor_tensor(
                                        out=qk_sb[:, mc, sl], in0=ps,
                                        in1=rstd_rep[:, sl], op=MULT)
                                    nc.vector.tensor_scalar_add(
                                        out=qk_sb[:, mc, sl],
                                        in0=qk_sb[:, mc, sl],
                                        scalar1=qkvb[:, mc, b:b + 1])


                    # rope in place: chunk pairs (c, c+4) for q and k
                    with tc.tile_pool(name=f"rt{b}", bufs=2) as prt:
                        for c in (0, 1, 2, 3, 8, 9, 10, 11):
                            ce, co = c, c + 4
                            t1 = prt.tile([128, NTOK], F32, tag="t1")
                            t2 = prt.tile([128, NTOK], F32, tag="t2")
                            t3 = prt.tile([128, NTOK], F32, tag="t3")
                            nc.vector.tensor_mul(t1, qk_sb[:, ce, :], cos4)
                            nc.vector.tensor_mul(t2, qk_sb[:, co, :], sin4)
                            nc.vector.tensor_mul(t3, qk_sb[:, ce, :], sin4)
                            nc.vector.tensor_mul(qk_sb[:, co, :],
                                                 qk_sb[:, co, :], cos4)
                            nc.vector.tensor_sub(qk_sb[:, ce, :], t1, t2)
                            nc.vector.tensor_add(qk_sb[:, co, :],
                                                 qk_sb[:, co, :], t3)

                    # ---- attention ----
                    with tc.tile_pool(name=f"ot{b}", bufs=1) as pot:
                        ot_sb = pot.tile([128, 8, NTOK], F32R, tag="ot")
                        with tc.tile_pool(name=f"pt{b}", bufs=12) as ppt, \
                             tc.tile_pool(name=f"rc{b}", bufs=2) as prc, \
                             tc.tile_pool(name=f"ps3_{b}", bufs=4,
                                          space="PSUM") as ps3, \
                             tc.tile_pool(name=f"pso{b}", bufs=2,
                                          space="PSUM") as pso:
                            for h in range(HEADS):
                                m = h % 4
                                pr = slice(32 * m, 32 * (m + 1))
                                ce, co = h // 4, 4 + h // 4
                                ke, ko = 8 + h // 4, 12 + h // 4
                                for tqc in range(2):
                                    sl = slice(512 * tqc, 512 * (tqc + 1))
                                    pts = []
                                    for tkt in range(TT):
                                        tk = slice(128 * tkt, 128 * (tkt + 1))
                                        ps = ps3.tile([128, 512], F32, tag="s")
                                        nc.tensor.matmul(
                                            ps, qk_sb[pr, ke, tk],
                                            qk_sb[pr, ce, sl],
                                            start=True, stop=False,
                                            tile_position=(32 * m, 0))
                                        nc.tensor.matmul(
                                            ps, qk_sb[pr, ko, tk],
                                            qk_sb[pr, co, sl],
                                            start=False, stop=True,
                                            tile_position=(32 * m, 0))
                                        pt = ppt.tile([128, 512], F32R, tag="pt")
                                        nc.scalar.activation(
                                            out=pt, in_=ps, func=EXP,
                                            scale=HD ** -0.5)
                                        pts.append(pt)
                                    ps_o = pso.tile([HD + 1, 512], F32, tag="o")
                                    for tkt in range(TT):
                                        nc.tensor.matmul(
                                            ps_o, v_sb[:, tkt, h, :], pts[tkt],
                                            start=(tkt == 0), stop=(tkt == TT - 1))
                                    rr = prc.tile([1, 512], F32, tag="rr")
                                    nc.vector.reciprocal(rr, ps_o[HD:HD + 1, :])
                                    rp = prc.tile([HD, 512], F32, tag="rp")
                                    nc.gpsimd.partition_broadcast(rp, rr)
                                    if h % 2 == 0:
                                        nc.vector.tensor_tensor(
                                            out=ot_sb[0:HD, h // 2, sl],
                                            in0=ps_o[0:HD, :], in1=rp, op=MULT)
                                    else:
                                        osh = prc.tile([HD, 512], F32R, tag="osh")
                                        nc.vector.tensor_tensor(
                                            out=osh, in0=ps_o[0:HD, :],
                                            in1=rp, op=MULT)
                                        nc.sync.dma_start(
                                            out=ot_sb[HD:128, h // 2, sl],
                                            in_=osh)

                        # ---- out projection ----
                        with tc.tile_pool(name=f"po{b}", bufs=9) as pwo, \
                             tc.tile_pool(name=f"ob{b}", bufs=2) as pob, \
                             tc.tile_pool(name=f"ps4_{b}", bufs=2,
                                          space="PSUM") as ps4:
                            for doutc in range(2):
                                wts = []
                                for jc in range(8):
                                    wt = pwo.tile([128, 512], F32R, tag="wo2")
                                    nc.sync.dma_start(
                                        out=wt,
                                        in_=wo_d[128 * jc:128 * (jc + 1),
                                                 512 * doutc:512 * (doutc + 1)])
                                    wts.append(wt)
                                for tt in range(TT):
                                    ps = ps4.tile([128, 512], F32, tag="out")
                                    for jc in range(8):
                                        nc.tensor.matmul(
                                            ps, ot_sb[:, jc, 128 * tt:128 * (tt + 1)],
                                            wts[jc], start=(jc == 0), stop=False)
                                    nc.tensor.matmul(
                                        ps, ones_r,
                                        bias_row[b][:, 512 * doutc:512 * (doutc + 1)],
                                        start=False, stop=True)
                                    ob = pob.tile([128, 512], F32, tag="ob")
                                    nc.vector.tensor_copy(ob, ps)
                                    nc.sync.dma_start(
                                        out=out_d[b, 128 * tt:128 * (tt + 1),
                                                  512 * doutc:512 * (doutc + 1)],
                                        in_=ob)
    nc.finalize()
    return nc


def _rope_tables():
    theta = 1.0 / (10000 ** (np.arange(0, 32, 2, dtype=np.float64)[:16] / 32))
    idx = np.arange(NTOK, dtype=np.float64)
    x_pos, y_pos = idx % 32, idx // 32
    freqs = np.concatenate([x_pos[:, None] * theta[None, :],
                            y_pos[:, None] * theta[None, :]], axis=-1)  # [n, 32]
    cos = np.cos(freqs).astype(np.float32)
    sin = np.sin(freqs).astype(np.float32)
    sel = np.arange(128) % 32
    return np.ascontiguousarray(cos.T[sel, :]), np.ascontiguousarray(sin.T[sel, :])


def kernel(x, t, norm_w, mod_w, qkv_w, wo_w):
    global LAST_EXEC_NS
    x = np.asarray(x, dtype=np.float32)
    t = np.asarray(t, dtype=np.float32)
    norm_w = np.asarray(norm_w, dtype=np.float32)
    mod_w = np.asarray(mod_w, dtype=np.float32)
    qkv_w = np.asarray(qkv_w, dtype=np.float32)
    wo_w = np.asarray(wo_w, dtype=np.float32)

    nw = np.where(norm_w == 0.0, 1.0, norm_w).astype(np.float32)
    qkv_wf = qkv_w * norm_w[None, :]
    q_even = [h * 192 + 2 * i for h in range(HEADS) for i in range(32)]
    q_odd = [h * 192 + 2 * i + 1 for h in range(HEADS) for i in range(32)]
    k_even = [h * 192 + 64 + 2 * i for h in range(HEADS) for i in range(32)]
    k_odd = [h * 192 + 64 + 2 * i + 1 for h in range(HEADS) for i in range(32)]
    perm_qk = q_even + q_odd + k_even + k_odd
    perm_v = [h * 192 + 128 + d for h in range(HEADS) for d in range(HD)]
    wqk = np.ascontiguousarray(qkv_wf[perm_qk, :].T)
    wv = np.ascontiguousarray(qkv_wf[perm_v, :].T)
    wo = np.ascontiguousarray(wo_w.T)
    mw = mod_w.copy()
    mw[DIM:, :] = mw[DIM:, :] / nw[:, None]
    mw = np.ascontiguousarray(mw.T)
    tT = np.ascontiguousarray(t.T)
    cos4, sin4 = _rope_tables()

    if "nc" not in _CACHE:
        _CACHE["nc"] = _build()
    nc = _CACHE["nc"]

    in_maps = []
    for c in range(NCORES):
        xs = x[BPC * c:BPC * (c + 1)]
        in_maps.append({
            "xT": np.ascontiguousarray(xs.transpose(0, 2, 1)),
            "tT": np.ascontiguousarray(tT[:, BPC * c:BPC * (c + 1)]),
            "wqk": wqk, "wv": wv, "wo": wo, "mw": mw,
            "cos4": cos4, "sin4": sin4,
        })
    trace = TRACE
    if trace:
        try:
            from antenv.axon_hooks import get_axon_ntff_profile_hook  # noqa: F401
        except Exception:
            trace = False
    res = run_bass_kernel_spmd(nc, in_maps, core_ids=list(range(NCORES)),
                               trace=trace)
    LAST_EXEC_NS = res.exec_time_ns
    out = np.concatenate([res.results[c]["out"] for c in range(NCORES)], axis=0)
    return out.astype(np.float32)
